# revision 11
# baseline (speedup 1.0000x reference)
"""ConvCaps (nn_ConvCaps_34995393528409) Trainium2 Bass kernel.

Math: out[b,h,w,x,y,o,m,n] = sum_i poses[b,h+x,w+y,i,m,n] * kernel[x,y,i,o,m,n]

Strategy ("Z-trick"):
  Z[b,p,x,y,o,m,n] = sum_i poses[b,p,i,m,n] * kernel[x,y,i,o,m,n]   (p = every
  input position). Then out[b,h,w,...] = Z[b,(h+x,w+y),x,y,...] is a pure
  re-indexing done by shifted-rectangle DMA writes, and the HBM layout of each
  output position's (x,y,o,m,n) block (4608 f32) is contiguous.

Per core (batch-sharded, B_LOCAL=4):
  - SBUF partition dim = q = b*20 + pw  (80 of 128) so each (b,x,y) scatter is
    a single contiguous partition range [b*20+y, +18).
  - Matmul per (ph, mn): stationary = poses [K=32 i, M=80 q], moving =
    kernel [K=32 i, N<=288 (x,y,o)], fp32, 4x PE row-tiling over mn groups
    (tile_position=(32r,0), mn = r*4+j).
  - PSUM [80, 288] evacuated by DVE/ACT with a stride-16 interleave so each
    partition's free dim becomes the HBM-contiguous (x,y,o,m,n) block.
  - 9 scatter DMAs per 4-row band write 2KB-contiguous runs straight into the
    output tensor.
"""

import os

import numpy as np

import concourse.bass as bass
import concourse.tile as tile
from concourse import bacc, mybir
from concourse.vector_clock import ScopedClock

F32 = mybir.dt.float32

N_CORES = 8
B_LOCAL = 4  # 32 / 8
HW20 = 20
OHW = 18
NI = 32
NO = 32
NMN = 16
NXY = 9
XYO = NXY * NO  # 288
BLK = XYO * NMN  # 4608 floats per (b,p) output block
PH_PER_BAND = 4
N_BANDS = 5
NQ = HW20 * B_LOCAL  # 80 partitions used
ZROW = PH_PER_BAND * BLK  # z free floats per partition per band

# output strides (elements) for the full per-core output [4,18,18,9,512]
OS_B = OHW * OHW * NXY * 512  # 1492992
OS_H = OHW * NXY * 512  # 82944
OS_W = NXY * 512  # 4608


def _patch_tile_drain():
    """This walrus build rejects >1 sync-wait on the Tile kernel-tail Drain;
    split the waits across a chain of drains."""
    if getattr(tile.TileContext, "_convcaps_drain_patch", False):
        return

    def _drain_and_barrier(self, tick_clock, wait_clock):
        drain_inst = self.nc.sync.drain()
        wait_clock.add_sem_waits(
            drain_inst.ins, ScopedClock({None: tick_clock.global_clock})
        )
        si = drain_inst.ins.sync_info
        w = list(si.on_wait or []) if si is not None else []
        if len(w) > 1:
            drain_inst.ins.sync_info = mybir.SyncInfo(
                on_wait=w[:1], on_update=list(si.on_update or [])
            )
            for x in w[1:]:
                extra = self.nc.sync.drain()
                extra.ins.sync_info = mybir.SyncInfo(on_wait=[x], on_update=[])
        self.nc.all_engine_barrier()
        assert self.sems is not None
        popped = self.nc._tile_sem_poison_stack.pop()
        assert popped is self._sem_poison
        self.nc.clear_and_free_semaphores(list(self.sems.allocated().values()))
        self.nc.all_engine_barrier()

    tile.TileContext._drain_and_barrier = _drain_and_barrier
    tile.TileContext._convcaps_drain_patch = True


def _build_nc(n_bands=N_BANDS):
    _patch_tile_drain()
    nc = bacc.Bacc("TRN2", target_bir_lowering=False, num_devices=N_CORES)

    # [r, i, (j, ph, q)] ; free idx = j*1600 + ph*80 + q ; q = pw*4 + b
    poses_d = nc.declare_dram_parameter(
        "poses_t", [4, NI, 4 * HW20 * NQ], F32, isOutput=False
    )
    # [r, i, (j, xy, o)] ; free idx = j*288 + xy*32 + o
    kern_d = nc.declare_dram_parameter(
        "kern_t", [4, NI, 4 * XYO], F32, isOutput=False
    )
    out_d = nc.declare_dram_parameter(
        "out", [B_LOCAL, OHW, OHW, NXY, 512], F32, isOutput=True
    )

    pose_sb = nc.alloc_sbuf_tensor("pose_sb", [128, 4 * HW20 * NQ], F32)
    kern_sb = nc.alloc_sbuf_tensor("kern_sb", [128, 4 * XYO], F32)
    z_sb = [nc.alloc_sbuf_tensor(f"z{i}", [128, ZROW], F32) for i in range(2)]

    pose_f = 4 * HW20 * NQ  # pose_sb free width
    kern_f = 4 * XYO

    with tile.TileContext(nc) as tc:
        for r in range(4):
            nc.sync.dma_start(
                pose_sb.ap()[32 * r : 32 * r + 32, :], poses_d.ap()[r]
            )
            nc.sync.dma_start(
                kern_sb.ap()[32 * r : 32 * r + 32, :], kern_d.ap()[r]
            )

        with tc.tile_pool(name="psum", bufs=8, space="PSUM") as pp:
            for band in range(n_bands):
                z = z_sb[band % 2]
                for phl in range(PH_PER_BAND):
                    ph = band * PH_PER_BAND + phl
                    x0 = max(0, ph - (OHW - 1))
                    x1 = min(2, ph)
                    n_xyo = (x1 - x0 + 1) * 3 * NO  # valid (x,y,o) span
                    for j in range(4):
                        for r in range(4):
                            mn = r * 4 + j
                            ps = pp.tile([128, XYO], F32, name="ps", tag="ps")
                            nc.tensor.matmul(
                                ps[0:NQ, 0:n_xyo],
                                pose_sb.ap()[
                                    32 * r : 32 * r + 32,
                                    j * (HW20 * NQ) + ph * NQ : j * (HW20 * NQ)
                                    + ph * NQ
                                    + NQ,
                                ],
                                kern_sb.ap()[
                                    32 * r : 32 * r + 32,
                                    j * XYO + x0 * 96 : j * XYO + (x1 + 1) * 96,
                                ],
                                start=True,
                                stop=True,
                                tile_position=(32 * r, 0),
                            )
                            dst = bass.AP(
                                z,
                                phl * BLK + x0 * 96 * NMN + mn,
                                [[ZROW, NQ], [NMN, n_xyo]],
                            )
                            if r == 2 or (r == 3 and j % 2 == 0):
                                nc.scalar.copy(dst, ps[0:NQ, 0:n_xyo])
                            else:
                                nc.vector.tensor_copy(dst, ps[0:NQ, 0:n_xyo])

                # scatter this band into the output
                for x in range(3):
                    ph0 = max(x, band * PH_PER_BAND)
                    ph_end = min(x + OHW, band * PH_PER_BAND + PH_PER_BAND)
                    nh = ph_end - ph0
                    if nh <= 0:
                        continue
                    phl0 = ph0 - band * PH_PER_BAND
                    for y in range(3):
                        xy = x * 3 + y
                        for b in range(B_LOCAL):
                            src = bass.AP(
                                z,
                                (b * HW20 + y) * ZROW + phl0 * BLK + xy * 512,
                                [[ZROW, OHW], [BLK, nh], [1, 512]],
                            )
                            dst = bass.AP(
                                out_d,
                                b * OS_B + (ph0 - x) * OS_H + xy * 512,
                                [[OS_W, OHW], [OS_H, nh], [1, 512]],
                            )
                            nc.sync.dma_start(dst, src)
    nc.finalize()
    return nc


_NC_CACHE = None


def _get_nc():
    global _NC_CACHE
    if _NC_CACHE is None:
        _NC_CACHE = _build_nc(
            int(os.environ.get("CONVCAPS_BANDS", str(N_BANDS)))
        )
    return _NC_CACHE


def _prep_poses(shard: np.ndarray) -> np.ndarray:
    # shard: (4, 20, 20, 32, 4, 4) -> [r, i, (j, ph, b*20+pw)]
    a = shard.reshape(B_LOCAL, HW20, HW20, NI, NMN)
    a = a.transpose(4, 3, 1, 0, 2)  # [mn, i, ph, b, pw]
    a = a.reshape(4, 4, NI, HW20, B_LOCAL, HW20)  # [r, j, i, ph, b, pw]
    a = a.transpose(0, 2, 1, 3, 4, 5)  # [r, i, j, ph, b, pw]
    return np.ascontiguousarray(a.reshape(4, NI, 4 * HW20 * NQ), dtype=np.float32)


def _prep_kernel(kern: np.ndarray) -> np.ndarray:
    # kern: (3, 3, 32, 32, 4, 4) -> [r, i, (j, xy, o)]
    a = kern.reshape(3, 3, NI, NO, NMN)
    a = a.transpose(4, 2, 0, 1, 3)  # [mn, i, x, y, o]
    a = a.reshape(4, 4, NI, 3, 3, NO)  # [r, j, i, x, y, o]
    a = a.transpose(0, 2, 1, 3, 4, 5)  # [r, i, j, x, y, o]
    return np.ascontiguousarray(a.reshape(4, NI, 4 * XYO), dtype=np.float32)


LAST_RESULTS = None  # set when CONVCAPS_TRACE=1, for test harness introspection


def kernel(**inputs) -> np.ndarray:
    from concourse.bass_utils import run_bass_kernel_spmd

    poses = np.asarray(inputs["poses"], dtype=np.float32)
    kern = np.asarray(inputs["kernel"], dtype=np.float32)

    nc = _get_nc()
    kern_t = _prep_kernel(kern)
    in_maps = []
    for c in range(N_CORES):
        shard = poses[c * B_LOCAL : (c + 1) * B_LOCAL]
        in_maps.append({"poses_t": _prep_poses(shard), "kern_t": kern_t})

    trace = os.environ.get("CONVCAPS_TRACE", "0") == "1"
    res = run_bass_kernel_spmd(
        nc, in_maps, core_ids=list(range(N_CORES)), trace=trace
    )
    if trace:
        global LAST_RESULTS
        LAST_RESULTS = res

    out = np.concatenate(
        [
            r["out"].reshape(B_LOCAL, OHW, OHW, 3, 3, NO, 4, 4)
            for r in res.results
        ],
        axis=0,
    )
    return out
